# revision 1
# baseline (speedup 1.0000x reference)
"""Trainium2 Bass kernel for nn_DinoText (retrieval_knn).

Computation (reference):
    t = l2norm(tanh(textual @ W.T + b))              [B, Dd]
    v = l2norm(visual, axis=-1)                      [B, P, Dd]
    sims = einsum('ik,ijk->ij', t, v); softmax; argmax -> idx  [B]
    v_best = v[b, idx[b]]                            [B, Dd]
    out = t @ v_best.T                               [B, B]

Strategy: data-parallel over batch across 8 NeuronCores (128 images each).

Layout inversion vs the straightforward stream: partitions hold
(patch-bank q in 0..3, image i in 0..31) for a 32-image group, and the
256 patches of each image stream through the FREE dim in 2 MiB tiles
(partition (q,i) sees patches 64q..64q+63 of image 32g+i).  Each
partition's text row is materialized once per group by a tiny PE
matmul with a replication matrix (t_norm rows are already
image-per-partition), so the per-image PE broadcast of t — the
dominant cost of the naive stream — disappears entirely:
  - VectorE:  scalar_tensor_tensor mult + accum  -> per-patch dots
  - ScalarE:  Square + accum_out                 -> per-patch sq.norms
softmax is monotonic so argmax(softmax(s)) == argmax(s); the cosine
score s/sqrt(n) is compared via the monotone transform u = s*|s|/n
(division-free sign-preserving square) so no sqrt is needed.
Per group: free-dim max_with_indices + a 4-way cross-bank tournament
give the winning patch; winners are re-fetched with an indirect DMA,
normalized, and AllGathered (overlapping the next group's stream).
The final [128,1024] x [1024,256]-per-group matmuls run on TensorE.
"""

import numpy as np

try:
    import concourse.bass as bass
except ImportError:  # toolchain lives in /opt in this container
    import sys

    for _p in ("/opt/pypackages", "/opt/trn_rl_repo"):
        if _p not in sys.path:
            sys.path.insert(0, _p)
    import concourse.bass as bass

import concourse.bacc as bacc
import concourse.mybir as mybir
import concourse.tile as tile
from concourse.bass_utils import run_bass_kernel_spmd
from concourse.masks import make_identity

NCORES = 8
B, P, DD, DC = 1024, 256, 1024, 512
BS = B // NCORES  # images per core
G = 32            # images per group (and per AllGather)
NG = BS // G      # groups per core
QB = 4            # patch banks (partition = 32*q + i)
PB = P // QB      # patches per bank (free-dim columns per group)
PPT = 4           # patches per DMA tile (2 MiB tiles)
TPG = PB // PPT   # tiles per group

AF = mybir.ActivationFunctionType
ALU = mybir.AluOpType
F32 = mybir.dt.float32
I32 = mybir.dt.int32
U32 = mybir.dt.uint32


def _build_kernel(tc, v_d, v3_d, wt_d, xt_d, b_d, o_d):
    nc = tc.nc
    import os as _os
    from contextlib import ExitStack

    ctx = ExitStack()
    # allocated first: its SBUF zone must not overlap the prep pools, or the
    # first v prefetches would wait for the prep-zone release
    vpool = ctx.enter_context(tc.tile_pool(name="vload", bufs=7))
    const = ctx.enter_context(tc.tile_pool(name="const", bufs=1))
    persist = ctx.enter_context(tc.tile_pool(name="persist", bufs=1))
    t4pool = ctx.enter_context(tc.tile_pool(name="t4", bufs=1))
    psum_tp = ctx.enter_context(tc.tile_pool(name="pstp", bufs=2, space="PSUM"))
    psum_s = ctx.enter_context(tc.tile_pool(name="pss", bufs=2, space="PSUM"))
    psum_n = ctx.enter_context(tc.tile_pool(name="psn", bufs=2, space="PSUM"))
    dram = ctx.enter_context(tc.tile_pool(name="dram", bufs=1, space="DRAM"))

    # ---- constants -------------------------------------------------------
    ident = const.tile([128, 128], F32, tag="ident")
    make_identity(nc, ident[:])
    ones_col = const.tile([1, 128], F32, tag="ones_col")
    nc.vector.memset(ones_col[:], 1.0)

    # selg[g][k, m] = 1 iff k == 32g + m % 32: K=128 selection matrix whose
    # matmul with t_norm replicates rows 32g..32g+31 to all 4 q-banks
    # (full-K form keeps every matmul operand at base partition 0)
    selg = []
    for g in range(NG):
        s = const.tile([128, 128], F32, tag=f"sel{g}", name=f"sel{g}")
        nc.gpsimd.memset(s[:], 0.0)
        for q in range(QB):
            nc.vector.tensor_copy(
                s[g * G : (g + 1) * G, q * G : (q + 1) * G], ident[0:G, 0:G]
            )
        selg.append(s)

    # bank_base[0, 32q+i] = 64*q  (patch-bank offset per partition slot)
    bank_base = const.tile([1, 128], F32, tag="bank_base")
    for q in range(QB):
        nc.vector.memset(bank_base[:, q * G : (q + 1) * G], float(PB * q))

    rowbase = []
    for g in range(NG):
        # rowbase[g][0, i] = (G*g + i) * 256: patch-row base of image G*g+i
        rbi = const.tile([1, G], I32, tag=f"rbi{g}", name=f"rbi{g}")
        nc.gpsimd.iota(
            rbi[:], pattern=[[P, G]], base=G * g * P, channel_multiplier=0
        )
        rb = const.tile([1, G], F32, tag=f"rb{g}", name=f"rb{g}")
        nc.vector.tensor_copy(rb[:], rbi[:])
        rowbase.append(rb)

    # ---- phase 0: t_norm = l2norm(tanh(x @ W.T + b)) ---------------------
    t_norm = persist.tile([128, DD], F32, tag="t_norm")
    tT = [persist.tile([128, 128], F32, tag=f"tT{k}", name=f"tT{k}") for k in range(8)]

    with tc.tile_pool(name="prep", bufs=2) as prep, tc.tile_pool(name="wtp", bufs=1) as wtp:
        # W.T and x.T are marshaled on the host, so the contraction operands
        # load straight into SBUF with no on-chip transposes
        wT = [wtp.tile([128, DD], F32, tag=f"wT{j}", name=f"wT{j}") for j in range(4)]
        xT = [wtp.tile([128, 128], F32, tag=f"xT{j}", name=f"xT{j}") for j in range(4)]
        for j in range(4):
            nc.sync.dma_start(out=wT[j][:], in_=wt_d[j * 128 : (j + 1) * 128, :])
            nc.sync.dma_start(out=xT[j][:], in_=xt_d[j * 128 : (j + 1) * 128, :])

        bsb = const.tile([1, DD], F32, tag="bsb")
        nc.sync.dma_start(out=bsb[:], in_=b_d[:, :])

        t_sb = prep.tile([128, DD], F32, tag="t_sb")
        for h in range(2):
            tp_ps = psum_s.tile([128, 512], F32, tag="tps")
            for j in range(4):
                nc.tensor.matmul(
                    out=tp_ps[:],
                    lhsT=xT[j][:],
                    rhs=wT[j][:, h * 512 : (h + 1) * 512],
                    start=(j == 0),
                    stop=False,
                )
            nc.tensor.matmul(
                out=tp_ps[:],
                lhsT=ones_col[:],
                rhs=bsb[:, h * 512 : (h + 1) * 512],
                start=False,
                stop=True,
            )
            nc.scalar.activation(
                out=t_sb[:, h * 512 : (h + 1) * 512], in_=tp_ps[:], func=AF.Tanh
            )

        tn2 = const.tile([128, 1], F32, tag="tn2")
        tscr = prep.tile([128, DD], F32, tag="tscr")
        nc.vector.scalar_tensor_tensor(
            out=tscr[:],
            in0=t_sb[:],
            scalar=0.0,
            in1=t_sb[:],
            op0=ALU.bypass,
            op1=ALU.mult,
            accum_out=tn2[:],
        )
        tinv = const.tile([128, 1], F32, tag="tinv")
        nc.vector.reciprocal(tinv[:], tn2[:])
        trsq = const.tile([128, 1], F32, tag="trsq")
        nc.scalar.activation(out=trsq[:], in_=tinv[:], func=AF.Sqrt)
        nc.scalar.activation(out=t_norm[:], in_=t_sb[:], func=AF.Copy, scale=trsq[:])

        for kc in range(8):
            pt = psum_tp.tile([128, 128], F32, tag="tp")
            nc.tensor.transpose(
                out=pt[:], in_=t_norm[:, kc * 128 : (kc + 1) * 128], identity=ident[:]
            )
            nc.vector.tensor_copy(tT[kc][:], pt[:])

    # ---- group accumulators / AllGather bounces --------------------------
    sims_g = [persist.tile([128, PB], F32, tag=f"sims{g}", name=f"sims{g}") for g in range(NG)]
    norms_g = [psum_n.tile([128, PB], F32, tag="norms", name=f"norms{g}") for g in range(NG)]
    # all groups' normalized winners accumulate here; ONE AllGather at the
    # end (a mid-stream collective starves the model DMA queues for its
    # whole barrier+transfer window — measured ~50us per group)
    ag_in = dram.tile([BS, DD], F32, tag="agin")
    _ag_space = "Local" if _os.environ.get("DINO_NO_COLLECTIVE") == "1" else "Shared"
    ag_out = dram.tile([B, DD], F32, tag="agout", addr_space=_ag_space)

    dscr = ctx.enter_context(tc.tile_pool(name="dscr", bufs=1))
    psum_sn = ctx.enter_context(tc.tile_pool(name="psnn", bufs=1, space="PSUM"))
    gp = ctx.enter_context(tc.tile_pool(name="gp", bufs=1))
    vbap = ctx.enter_context(tc.tile_pool(name="vba", bufs=1))
    ldp2 = ctx.enter_context(tc.tile_pool(name="ld2", bufs=2))
    sgp = ctx.enter_context(tc.tile_pool(name="sg", bufs=2))

    # v3 is host-marshaled so tile (g,t) is one contiguous [128, PPT*DD]
    # block with partition 32q+i = (patch-bank q, image G*g+i); the original
    # v stays resident only for the winner gather
    v_flat = v_d.rearrange("b p k -> (b p) k")

    sd = dscr.tile([128, DD], F32, tag="sd")
    # ScalarE is closer to PSUM: Square scratch there shaves ~50 cyc/op
    sn = psum_sn.tile([128, DD], F32, tag="sn")

    def process_group(g):
        """argmax of u = s*|s|/n over this group's 64 free-dim scores plus a
        4-way cross-bank tournament; gather + normalize winners, kick off
        the AllGather."""
        rn = gp.tile([128, PB], F32, tag="rn")
        nc.vector.reciprocal(rn[:], norms_g[g][:])
        sneg = gp.tile([128, PB], F32, tag="sneg")
        nc.vector.tensor_scalar_mul(sneg[:], sims_g[g][:], -1.0)
        sabs = gp.tile([128, PB], F32, tag="sabs")
        nc.vector.tensor_tensor(sabs[:], sims_g[g][:], sneg[:], op=ALU.max)
        rat = gp.tile([128, PB], F32, tag="rat")
        nc.vector.tensor_tensor(rat[:], sims_g[g][:], rn[:], op=ALU.mult)
        u = gp.tile([128, PB], F32, tag="u")
        nc.vector.tensor_tensor(u[:], rat[:], sabs[:], op=ALU.mult)

        mx = gp.tile([128, 8], F32, tag="mx")
        mi = gp.tile([128, 8], U32, tag="mi")
        nc.vector.max_with_indices(out_max=mx[:], out_indices=mi[:], in_=u[:])
        mif = gp.tile([128, 1], F32, tag="mif")
        nc.vector.tensor_copy(mif[:], mi[:, 0:1])

        # bring per-partition maxes/indices onto partition 0 via PE
        # transposes (engines can't mix SBUF base partitions)
        ptm = psum_tp.tile([1, 128], F32, tag="tp")
        nc.tensor.transpose(out=ptm[:], in_=mx[:, 0:1], identity=ident[:])
        mxT = gp.tile([1, 128], F32, tag="mxT")
        nc.vector.tensor_copy(mxT[:], ptm[:])
        pti = psum_tp.tile([1, 128], F32, tag="tp")
        nc.tensor.transpose(out=pti[:], in_=mif[:], identity=ident[:])
        giT = gp.tile([1, 128], F32, tag="giT")
        # global patch index = bank-local index + 64*q
        nc.vector.tensor_tensor(giT[:], pti[:], bank_base[:], op=ALU.add)

        bm = gp.tile([1, G], F32, tag="bm")
        nc.vector.tensor_copy(bm[:], mxT[0:1, 0:G])
        bi = gp.tile([1, G], F32, tag="bi")
        nc.vector.tensor_copy(bi[:], giT[0:1, 0:G])
        for q in range(1, QB):
            sl = slice(q * G, (q + 1) * G)
            gt = gp.tile([1, G], U32, tag="gt")
            nc.vector.tensor_tensor(gt[:], mxT[0:1, sl], bm[:], op=ALU.is_gt)
            nc.vector.copy_predicated(bm[:], gt[:], mxT[0:1, sl])
            nc.vector.copy_predicated(bi[:], gt[:], giT[0:1, sl])

        # winner HBM row = (G*g + i)*256 + best patch
        grow = gp.tile([1, G], F32, tag="grow")
        nc.vector.tensor_tensor(grow[:], bi[:], rowbase[g][:], op=ALU.add)
        ptg = psum_tp.tile([G, 1], F32, tag="tp")
        nc.tensor.transpose(out=ptg[:], in_=grow[:], identity=ident[0:1, 0:1])
        gidxf = gp.tile([G, 1], F32, tag="gidxf")
        nc.vector.tensor_copy(gidxf[:], ptg[:])
        gidx = gp.tile([G, 1], I32, tag="gidx")
        nc.vector.tensor_copy(gidx[:], gidxf[:])

        vb = gp.tile([G, DD], F32, tag="vb")
        if _os.environ.get("DINO_NO_GATHER") == "1":
            # debug: fixed gather (patch 0 of each image) — wrong result,
            # exercises everything but the indirect DMA
            nc.sync.dma_start(out=vb[:], in_=v_d[G * g : G * (g + 1), 0, :])
        else:
            nc.gpsimd.indirect_dma_start(
                out=vb[:],
                out_offset=None,
                in_=v_flat,
                in_offset=bass.IndirectOffsetOnAxis(ap=gidx[:], axis=0),
            )
        vbs = gp.tile([G, DD], F32, tag="vbs")
        nb2 = gp.tile([G, 1], F32, tag="nb2")
        nc.scalar.activation(out=vbs[:], in_=vb[:], func=AF.Square, accum_out=nb2[:])
        nbr = gp.tile([G, 1], F32, tag="nbr")
        nc.vector.reciprocal(nbr[:], nb2[:])
        nbs = gp.tile([G, 1], F32, tag="nbs")
        nc.scalar.activation(out=nbs[:], in_=nbr[:], func=AF.Sqrt)
        vbn = gp.tile([G, DD], F32, tag="vbn")
        nc.scalar.activation(out=vbn[:], in_=vb[:], func=AF.Copy, scale=nbs[:])
        # issue on the gpsimd (SWDGE) queue: the SP sequencer carries the
        # main v stream and must not block on this chain
        nc.gpsimd.dma_start(out=ag_in[g * G : (g + 1) * G, :], in_=vbn[:])

    def finale():
        """one AllGather of all 128 normalized winners, then per-128-column
        chunk: load, transpose, and the S matmul straight into the output
        (ag_out row order == global image order, so no column permute)."""
        if _os.environ.get("DINO_NO_COLLECTIVE") == "1":
            # debug: replicate local shard into all slots — wrong result
            for cc in range(NCORES):
                nc.gpsimd.dma_start(
                    out=ag_out[cc * BS : (cc + 1) * BS, :], in_=ag_in[:]
                )
        else:
            nc.gpsimd.collective_compute(
                "AllGather",
                ALU.bypass,
                replica_groups=[list(range(NCORES))],
                ins=[ag_in[:].opt()],
                outs=[ag_out[:].opt()],
            )
        for q2 in range(B // 128):
            ld = ldp2.tile([128, DD], F32, tag="ld")
            nc.gpsimd.dma_start(out=ld[:], in_=ag_out[q2 * 128 : (q2 + 1) * 128, :])
            vba = [
                vbap.tile([128, 128], F32, tag=f"vba{kc}", name=f"vba{kc}_{q2}")
                for kc in range(8)
            ]
            for kc in range(8):
                pt = psum_tp.tile([128, 128], F32, tag="tp")
                nc.tensor.transpose(
                    out=pt[:], in_=ld[:, kc * 128 : (kc + 1) * 128], identity=ident[:]
                )
                if kc % 2 == 0:
                    nc.vector.tensor_copy(vba[kc][:], pt[:])
                else:
                    nc.scalar.copy(vba[kc][:], pt[:])
            spg = psum_s.tile([128, 128], F32, tag="tps")
            for kc in range(8):
                nc.tensor.matmul(
                    out=spg[:],
                    lhsT=tT[kc][:],
                    rhs=vba[kc][:],
                    start=(kc == 0),
                    stop=(kc == 7),
                )
            s_g = sgp.tile([128, 128], F32, tag="sg")
            nc.scalar.activation(out=s_g[:], in_=spg[:], func=AF.Copy)
            nc.gpsimd.dma_start(out=o_d[:, q2 * 128 : (q2 + 1) * 128], in_=s_g[:])

    # ---- main stream -----------------------------------------------------
    # T4[g][32q+i, :] = t_norm[32g+i, :] via the replication matmuls
    t4s = []
    for g in range(NG):
        t4 = t4pool.tile([128, DD], F32, tag=f"t4_{g}", name=f"t4_{g}")
        for h in range(2):
            ps = psum_s.tile([128, 512], F32, tag="tps")
            nc.tensor.matmul(
                out=ps[:],
                lhsT=selg[g][:],
                rhs=t_norm[:, h * 512 : (h + 1) * 512],
                start=True,
                stop=True,
            )
            nc.vector.tensor_copy(t4[:, h * 512 : (h + 1) * 512], ps[:])
        t4s.append(t4)
    for g in range(NG):
        t4 = t4s[g]
        if g > 0:
            process_group(g - 1)
        for t in range(TPG):
            vt = vpool.tile([128, PPT * DD], F32, tag="vt")
            nc.sync.dma_start(out=vt[:], in_=v3_d[g, t])
            for c in range(PPT):
                col = t * PPT + c
                nc.vector.scalar_tensor_tensor(
                    out=sd[:],
                    in0=vt[:, c * DD : (c + 1) * DD],
                    scalar=0.0,
                    in1=t4[:],
                    op0=ALU.bypass,
                    op1=ALU.mult,
                    accum_out=sims_g[g][:, col : col + 1],
                )
                nc.scalar.activation(
                    out=sn[:],
                    in_=vt[:, c * DD : (c + 1) * DD],
                    func=AF.Square,
                    accum_out=norms_g[g][:, col : col + 1],
                )
    process_group(NG - 1)
    finale()

    ctx.close()


_CACHE = {}


def build():
    if "nc" in _CACHE:
        return _CACHE["nc"]
    nc = bacc.Bacc(
        "TRN2", target_bir_lowering=False, debug=False, num_devices=NCORES
    )
    v_d = nc.dram_tensor("v", [BS, P, DD], F32, kind="ExternalInput").ap()
    v3_d = nc.dram_tensor(
        "v3", [NG, TPG, 128, PPT * DD], F32, kind="ExternalInput"
    ).ap()
    wt_d = nc.dram_tensor("wt", [DC, DD], F32, kind="ExternalInput").ap()
    xt_d = nc.dram_tensor("xt", [DC, BS], F32, kind="ExternalInput").ap()
    b_d = nc.dram_tensor("bv", [1, DD], F32, kind="ExternalInput").ap()
    o_d = nc.dram_tensor("out", [BS, B], F32, kind="ExternalOutput").ap()
    with tile.TileContext(nc) as tc:
        _build_kernel(tc, v_d, v3_d, wt_d, xt_d, b_d, o_d)
    nc.compile()
    _CACHE["nc"] = nc
    return nc


def make_in_maps(visual_embedding, textual_embedding, W, b):
    in_maps = []
    for c in range(NCORES):
        sl = slice(c * BS, (c + 1) * BS)
        vs = np.asarray(visual_embedding[sl], dtype=np.float32)
        v3 = np.ascontiguousarray(
            vs.reshape(NG, G, QB, TPG, PPT, DD)
            .transpose(0, 3, 2, 1, 4, 5)
            .reshape(NG, TPG, 128, PPT * DD)
        )
        in_maps.append(
            {
                "v": np.ascontiguousarray(vs),
                "v3": v3,
                "wt": np.ascontiguousarray(np.asarray(W, dtype=np.float32).T),
                "xt": np.ascontiguousarray(
                    np.asarray(textual_embedding[sl], dtype=np.float32).T
                ),
                "bv": np.ascontiguousarray(b, dtype=np.float32).reshape(1, DD),
            }
        )
    return in_maps


def kernel(visual_embedding, textual_embedding, W, b, _trace=False, _tmpdir=None):
    nc = build()
    in_maps = make_in_maps(visual_embedding, textual_embedding, W, b)
    res = run_bass_kernel_spmd(
        nc, in_maps, list(range(NCORES)), trace=_trace, tmpdir=_tmpdir
    )
    out = np.concatenate([res.results[c]["out"] for c in range(NCORES)], axis=0)
    if _trace:
        kernel.last_exec_time_ns = res.exec_time_ns
        kernel.last_profile = res.profile_json
        iat = res.instructions_and_trace
        kernel.last_trace_path = iat[1] if iat else None
    return out

